# revision 23
# baseline (speedup 1.0000x reference)
"""Trainium2 Bass kernel for DiceLoss (hard-argmax dice, ignore background, mean).

Problem (hardcoded shapes):
  y_true: [16, 512, 512] int32 in [0, 8)
  y_pred: [16, 8, 512, 512] float32
  out   : scalar float32 = mean over classes 1..7 of
          (2*tp + eps) / (2*tp + fp + fn + eps)
        = (2*tp + eps) / (pred_cnt + gt_cnt + eps)

Strategy (8 NeuronCores, data-parallel over batch; 2 images/core):
  - Streams image planes as [128, 1024] chunks (contiguous HBM DMA).
  - ScalarE: f32->fp16 channel converts + int32->fp16 label convert.
  - DVE (all fp16, 2x/4x perf modes): 7-op pairwise max tree; per class
    pred_c = (ch[c] == m) via tensor_tensor is_equal written into a
    [128, 8, 129] layout whose group-col 0 holds ones; gt_c = (y == c)
    via tensor_scalar is_equal (4x mode, flat [128, 1024]).
  - PE: per (class, chunk, subtile) one matmul
        psum_c[:, 0:129] += gt_s^T @ [ones | pred_s]
    giving gt colsums in col 0 and tp on the shifted diagonal (7 psum
    banks, subtile-outer emission so consecutive matmuls hit different
    banks and pipeline); plus per (class, chunk) 2 one-hot-stationary
    colsum matmuls
        psum_b[0:7, 0:512] += onehot_c^T @ pred(4 groups of 128)
    accumulating pred counts for all 7 classes row-wise in one shared
    8th PSUM bank. Only the very first matmul into a bank sets start
    (start zeroes the whole bank).
  - Host: combines the 8 cores' exact f32 count sums into the dice mean.
"""

import numpy as np

EPS = 1e-05

N_CORES = 8
NB = 2          # batch images per core
C = 8           # classes
P = 128         # SBUF partitions
F = 1024        # free-dim elements per chunk
NCHUNK = 2      # chunks per image plane (512*512 = 2*128*1024)
CHUNKS = NB * NCHUNK
NSUB = F // 128  # 8 subtiles per chunk

_CACHED_NC = None


def build_bass():
    from contextlib import ExitStack

    import concourse.bacc as bacc
    import concourse.tile as tile
    from concourse import mybir

    AL = mybir.AluOpType
    ACT = mybir.ActivationFunctionType

    nc = bacc.Bacc(None, target_bir_lowering=False)

    yp = nc.dram_tensor(
        "yp", [NB, C, NCHUNK, P, F], mybir.dt.float32, kind="ExternalInput"
    )
    yt = nc.dram_tensor("yt", [NB, NCHUNK, P, F], mybir.dt.int32, kind="ExternalInput")
    # per class: [128, 129] A-region (col0 = gt colsums, diag = tp);
    # partition-major so one contiguous DMA ships all 7 classes
    a_out = nc.dram_tensor("a_out", [P, 7, 129], mybir.dt.float32, kind="ExternalOutput")
    # pred-count partial colsums: row c-1 = class c
    b_out = nc.dram_tensor("b_out", [7, 512], mybir.dt.float32, kind="ExternalOutput")

    with tile.TileContext(nc) as tc, ExitStack() as ctx:
        chpool = ctx.enter_context(tc.tile_pool(name="ch", bufs=2))
        chfpool = ctx.enter_context(tc.tile_pool(name="chf", bufs=2))
        tpool = ctx.enter_context(tc.tile_pool(name="tt", bufs=2))
        mtmp = ctx.enter_context(tc.tile_pool(name="mtmp", bufs=2))
        mpool = ctx.enter_context(tc.tile_pool(name="mx", bufs=2))
        gtpool = ctx.enter_context(tc.tile_pool(name="gt", bufs=2))
        # two fixed pred-tile sets (manual double buffer, ones cols set once)
        predpA = ctx.enter_context(tc.tile_pool(name="pdA", bufs=1))
        predpB = ctx.enter_context(tc.tile_pool(name="pdB", bufs=1))
        onesp = ctx.enter_context(tc.tile_pool(name="on", bufs=1))
        psump = ctx.enter_context(tc.tile_pool(name="psum", bufs=1, space="PSUM"))

        banks = [
            psump.tile([P, 512], mybir.dt.float32, name=f"bk{c}", tag=f"bk{c}")
            for c in range(7)
        ]
        bbank = psump.tile([P, 512], mybir.dt.float32, name="bb", tag="bb")

        # one-hot stationary columns: ohs[c-1][:, c-1] = 1 for class c
        ohs = []
        for c in range(7):
            t = onesp.tile([P, 7], mybir.dt.float16, name=f"oh{c}")
            nc.vector.memset(t, 0.0)
            nc.vector.memset(t[:, c:c + 1], 1.0)
            ohs.append(t)

        predA = [
            predpA.tile([P, NSUB, 129], mybir.dt.float16, name=f"pA{c}", tag=f"pA{c}")
            for c in range(1, C)
        ]
        predB = [
            predpB.tile([P, NSUB, 129], mybir.dt.float16, name=f"pB{c}", tag=f"pB{c}")
            for c in range(1, C)
        ]
        for t in predA + predB:
            nc.vector.memset(t[:, :, 0:1], 1.0)

        # Work list: (image, plane-chunk, col offset, width). The final plane
        # is processed in two half-width pieces so the last piece's serial
        # convert -> tree -> mask -> matmul chain (the kernel's tail, which
        # runs after the DMA window closes) is half as long.
        pieces = [
            (0, 0, 0, F), (0, 1, 0, F), (1, 0, 0, F),
            (1, 1, 0, F // 2), (1, 1, F // 2, F // 2),
        ]
        for pi, (n, j, off, W) in enumerate(pieces):
            first = pi == 0
            last = pi == len(pieces) - 1
            NS = W // 128
            preds = predA if pi % 2 == 0 else predB

            # Label DMA first: the label convert heads ScalarE's queue and
            # the gt masks head DVE's queue, so yt must never be the
            # straggler transfer. Channel dispatch split sync/gpsimd to
            # fill the DMA queues faster.
            tt_ = tpool.tile([P, F], mybir.dt.int32, name="t", tag="t")
            nc.sync.dma_start(out=tt_[:, 0:W], in_=yt[n, j][:, off:off + W])
            ch = []
            for c in range(C):
                tl = chpool.tile([P, F], mybir.dt.float32, name=f"ch{c}", tag=f"ch{c}")
                eng = nc.sync if c < 4 else nc.gpsimd
                eng.dma_start(out=tl[:, 0:W], in_=yp[n, c, j][:, off:off + W])
                ch.append(tl)

            # ---- ScalarE: converts (label first: unblocks gt masks) ----
            yf = tpool.tile([P, F], mybir.dt.float16, name="yf", tag="yf")
            nc.scalar.activation(out=yf[:, 0:W], in_=tt_[:, 0:W], func=ACT.Copy)
            chf = []
            for c in range(C):
                tf = chfpool.tile([P, F], mybir.dt.float16, name=f"cf{c}", tag=f"cf{c}")
                nc.scalar.activation(out=tf[:, 0:W], in_=ch[c][:, 0:W], func=ACT.Copy)
                chf.append(tf)

            # ---- DVE: gt masks first (depend only on yf, which converts
            # first — they overlap the channel converts) ----
            gts = []
            for c in range(1, C):
                gt = gtpool.tile([P, F], mybir.dt.float16, name=f"gt{c}", tag=f"gt{c}")
                nc.vector.tensor_scalar(
                    out=gt[:, 0:W], in0=yf[:, 0:W], scalar1=float(c), scalar2=0.0,
                    op0=AL.is_equal, op1=AL.add,
                )
                gts.append(gt)

            # ---- DVE: max tree (fp16 tensor_tensor, 2x) ----
            m01 = mtmp.tile([P, F], mybir.dt.float16, name="m01", tag="m01")
            nc.vector.tensor_max(m01[:, 0:W], chf[0][:, 0:W], chf[1][:, 0:W])
            m23 = mtmp.tile([P, F], mybir.dt.float16, name="m23", tag="m23")
            nc.vector.tensor_max(m23[:, 0:W], chf[2][:, 0:W], chf[3][:, 0:W])
            m45 = mtmp.tile([P, F], mybir.dt.float16, name="m45", tag="m45")
            nc.vector.tensor_max(m45[:, 0:W], chf[4][:, 0:W], chf[5][:, 0:W])
            m67 = mtmp.tile([P, F], mybir.dt.float16, name="m67", tag="m67")
            nc.vector.tensor_max(m67[:, 0:W], chf[6][:, 0:W], chf[7][:, 0:W])
            m0123 = mtmp.tile([P, F], mybir.dt.float16, name="m0123", tag="m01")
            nc.vector.tensor_max(m0123[:, 0:W], m01[:, 0:W], m23[:, 0:W])
            m4567 = mtmp.tile([P, F], mybir.dt.float16, name="m4567", tag="m45")
            nc.vector.tensor_max(m4567[:, 0:W], m45[:, 0:W], m67[:, 0:W])
            m = mpool.tile([P, F], mybir.dt.float16, name="m", tag="m")
            nc.vector.tensor_max(m[:, 0:W], m0123[:, 0:W], m4567[:, 0:W])

            # ---- per class: pred mask (tt is_equal, 2x) ----
            for c in range(1, C):
                px = preds[c - 1]
                nc.vector.tensor_tensor(
                    out=px[:, 0:NS, 1:129], in0=chf[c][:, 0:W], in1=m[:, 0:W],
                    op=AL.is_equal,
                )

            # ---- PE: diag MMs subtile-outer so consecutive MMs hit
            # different psum banks and pipeline; then shared-bank colsums ----
            for s in range(NS):
                for c in range(1, C):
                    nc.tensor.matmul(
                        banks[c - 1][:, 0:129],
                        lhsT=gts[c - 1][:, s * 128:(s + 1) * 128],
                        rhs=preds[c - 1][:, s, 0:129],
                        start=(first and s == 0),
                        stop=(last and s == NS - 1),
                        skip_group_check=True,
                    )
            # colsum MMs in groups of <=4 subtiles; the host sums every psum
            # column, so narrow pieces accumulating only cols [0:gw) is fine
            gstarts = list(range(0, NS, 4))
            for c in range(1, C):
                for gi, g0 in enumerate(gstarts):
                    ng = min(4, NS - g0)
                    nc.tensor.matmul(
                        bbank[0:7, 0:ng * 128],
                        lhsT=ohs[c - 1][:, 0:7],
                        rhs=preds[c - 1][:, g0:g0 + ng, 1:129],
                        start=(first and c == 1 and gi == 0),
                        stop=(last and c == C - 1 and gi == len(gstarts) - 1),
                        skip_group_check=True,
                    )

        # ---- readback: PSUM -> one SBUF tile (copies split DVE/ScalarE,
        # run in parallel) -> single DMA per output ----
        outp = ctx.enter_context(tc.tile_pool(name="out", bufs=1))
        oa = outp.tile([P, 7, 129], mybir.dt.float32, name="oa", tag="oa")
        for c in range(7):
            if c % 2 == 0:
                nc.vector.tensor_copy(out=oa[:, c, :], in_=banks[c][:, 0:129])
            else:
                nc.scalar.copy(out=oa[:, c, :], in_=banks[c][:, 0:129])
        nc.sync.dma_start(out=a_out[:, :, :], in_=oa)
        ob = outp.tile([7, 512], mybir.dt.float32, name="ob", tag="ob")
        nc.scalar.copy(out=ob, in_=bbank[0:7, 0:512])
        nc.sync.dma_start(out=b_out[:, :], in_=ob)

    nc.finalize()
    return nc


def _get_bass():
    global _CACHED_NC
    if _CACHED_NC is None:
        _CACHED_NC = build_bass()
    return _CACHED_NC


def make_in_maps(y_true, y_pred):
    yp = np.ascontiguousarray(np.asarray(y_pred, dtype=np.float32))
    yt = np.ascontiguousarray(np.asarray(y_true, dtype=np.int32))
    in_maps = []
    for i in range(N_CORES):
        yps = np.ascontiguousarray(yp[NB * i: NB * (i + 1)]).reshape(NB, C, NCHUNK, P, F)
        yts = np.ascontiguousarray(yt[NB * i: NB * (i + 1)]).reshape(NB, NCHUNK, P, F)
        in_maps.append({"yp": yps, "yt": yts})
    return in_maps


def epilogue(results):
    """Combine the 8 cores' exact f32 partial sums into the dice mean."""
    tp = np.zeros(7, dtype=np.float64)
    gt_cnt = np.zeros(7, dtype=np.float64)
    pred_cnt = np.zeros(7, dtype=np.float64)
    idx = np.arange(128)
    for r in results:
        a = np.asarray(r["a_out"], dtype=np.float64)   # [128, 7, 129]
        b = np.asarray(r["b_out"], dtype=np.float64)   # [7, 512]
        gt_cnt += a[:, :, 0].sum(axis=0)
        tp += a[idx, :, 1 + idx].sum(axis=0)
        pred_cnt += b.sum(axis=1)

    tp32 = tp.astype(np.float32)
    denom = (pred_cnt + gt_cnt).astype(np.float32)
    eps = np.float32(EPS)
    two = np.float32(2.0)
    dice = (two * tp32 + eps) / (denom + eps)
    return np.asarray(np.mean(dice, dtype=np.float32), dtype=np.float32)


def kernel(**inputs):
    from concourse.bass_utils import run_bass_kernel_spmd

    nc = _get_bass()
    in_maps = make_in_maps(inputs["y_true"], inputs["y_pred"])
    res = run_bass_kernel_spmd(nc, in_maps, core_ids=list(range(N_CORES)))
    return epilogue(res.results)


if __name__ == "__main__":
    rng = np.random.default_rng(0)
    y_true = rng.integers(0, C, size=(16, 512, 512)).astype(np.int32)
    y_pred = rng.standard_normal((16, C, 512, 512)).astype(np.float32)
    out = kernel(y_true=y_true, y_pred=y_pred)
    print("kernel output:", out)

    # numpy oracle
    pred_cls = np.argmax(y_pred, axis=1)
    tp = np.zeros(7); fp = np.zeros(7); fn = np.zeros(7)
    for c in range(1, 8):
        pm = pred_cls == c
        gm = y_true == c
        tp[c-1] = np.sum(pm & gm)
        fp[c-1] = np.sum(pm & ~gm)
        fn[c-1] = np.sum(~pm & gm)
    dice = (2*tp + EPS) / (2*tp + fp + fn + EPS)
    print("numpy oracle:", dice.mean())


# revision 24
# speedup vs baseline: 1.1888x; 1.1888x over previous
"""Trainium2 Bass kernel for DiceLoss (hard-argmax dice, ignore background, mean).

Problem (hardcoded shapes):
  y_true: [16, 512, 512] int32 in [0, 8)
  y_pred: [16, 8, 512, 512] float32
  out   : scalar float32 = mean over classes 1..7 of
          (2*tp + eps) / (2*tp + fp + fn + eps)
        = (2*tp + eps) / (pred_cnt + gt_cnt + eps)

Strategy (8 NeuronCores, data-parallel over batch; 2 images/core):
  - Streams image planes as [128, 1024] chunks (contiguous HBM DMA).
  - ScalarE: f32->fp16 channel converts + int32->fp16 label convert.
  - DVE (all fp16, 2x/4x perf modes): 7-op pairwise max tree; per class
    pred_c = (ch[c] == m) via tensor_tensor is_equal written into a
    [128, 8, 129] layout whose group-col 0 holds ones; gt_c = (y == c)
    via tensor_scalar is_equal (4x mode, flat [128, 1024]).
  - PE: per (class, chunk, subtile) one matmul
        psum_c[:, 0:129] += gt_s^T @ [ones | pred_s]
    giving gt colsums in col 0 and tp on the shifted diagonal (7 psum
    banks, subtile-outer emission so consecutive matmuls hit different
    banks and pipeline); plus per (class, chunk) 2 one-hot-stationary
    colsum matmuls
        psum_b[0:7, 0:512] += onehot_c^T @ pred(4 groups of 128)
    accumulating pred counts for all 7 classes row-wise in one shared
    8th PSUM bank. Only the very first matmul into a bank sets start
    (start zeroes the whole bank).
  - Host: combines the 8 cores' exact f32 count sums into the dice mean.
"""

import numpy as np

EPS = 1e-05

N_CORES = 8
NB = 2          # batch images per core
C = 8           # classes
P = 128         # SBUF partitions
F = 1024        # free-dim elements per chunk
NCHUNK = 2      # chunks per image plane (512*512 = 2*128*1024)
CHUNKS = NB * NCHUNK
NSUB = F // 128  # 8 subtiles per chunk

_CACHED_NC = None


def build_bass():
    from contextlib import ExitStack

    import concourse.bacc as bacc
    import concourse.tile as tile
    from concourse import mybir

    AL = mybir.AluOpType
    ACT = mybir.ActivationFunctionType

    nc = bacc.Bacc(None, target_bir_lowering=False)

    yp = nc.dram_tensor(
        "yp", [NB, C, NCHUNK, P, F], mybir.dt.float32, kind="ExternalInput"
    )
    yt = nc.dram_tensor("yt", [NB, NCHUNK, P, F], mybir.dt.int32, kind="ExternalInput")
    # per class: [128, 129] A-region (col0 = gt colsums, diag = tp);
    # partition-major so one contiguous DMA ships all 7 classes
    a_out = nc.dram_tensor("a_out", [P, 7, 129], mybir.dt.float32, kind="ExternalOutput")
    # pred-count partial colsums: row c-1 = class c
    b_out = nc.dram_tensor("b_out", [7, 512], mybir.dt.float32, kind="ExternalOutput")

    with tile.TileContext(nc) as tc, ExitStack() as ctx:
        chpool = ctx.enter_context(tc.tile_pool(name="ch", bufs=2))
        chfpool = ctx.enter_context(tc.tile_pool(name="chf", bufs=2))
        tpool = ctx.enter_context(tc.tile_pool(name="tt", bufs=2))
        mtmp = ctx.enter_context(tc.tile_pool(name="mtmp", bufs=2))
        mpool = ctx.enter_context(tc.tile_pool(name="mx", bufs=2))
        gtpool = ctx.enter_context(tc.tile_pool(name="gt", bufs=2))
        # two fixed pred-tile sets (manual double buffer, ones cols set once)
        predpA = ctx.enter_context(tc.tile_pool(name="pdA", bufs=1))
        predpB = ctx.enter_context(tc.tile_pool(name="pdB", bufs=1))
        onesp = ctx.enter_context(tc.tile_pool(name="on", bufs=1))
        psump = ctx.enter_context(tc.tile_pool(name="psum", bufs=1, space="PSUM"))

        banks = [
            psump.tile([P, 512], mybir.dt.float32, name=f"bk{c}", tag=f"bk{c}")
            for c in range(7)
        ]
        bbank = psump.tile([P, 512], mybir.dt.float32, name="bb", tag="bb")

        # one-hot stationary columns: ohs[c-1][:, c-1] = 1 for class c
        ohs = []
        for c in range(7):
            t = onesp.tile([P, 7], mybir.dt.float16, name=f"oh{c}")
            nc.vector.memset(t, 0.0)
            nc.vector.memset(t[:, c:c + 1], 1.0)
            ohs.append(t)

        predA = [
            predpA.tile([P, NSUB, 129], mybir.dt.float16, name=f"pA{c}", tag=f"pA{c}")
            for c in range(1, C)
        ]
        predB = [
            predpB.tile([P, NSUB, 129], mybir.dt.float16, name=f"pB{c}", tag=f"pB{c}")
            for c in range(1, C)
        ]
        for t in predA + predB:
            nc.vector.memset(t[:, :, 0:1], 1.0)

        # Work list: (image, plane-chunk, col offset, width). The final plane
        # is processed in two half-width pieces so the last piece's serial
        # convert -> tree -> mask -> matmul chain (the kernel's tail, which
        # runs after the DMA window closes) is half as long.
        pieces = [
            (0, 0, 0, F), (0, 1, 0, F), (1, 0, 0, F),
            (1, 1, 0, F // 2), (1, 1, F // 2, F // 2),
        ]
        for pi, (n, j, off, W) in enumerate(pieces):
            first = pi == 0
            last = pi == len(pieces) - 1
            NS = W // 128
            preds = predA if pi % 2 == 0 else predB

            # Label DMA first: the label convert heads ScalarE's queue and
            # the gt masks head DVE's queue, so yt must never be the
            # straggler transfer. Channel dispatch split sync/gpsimd to
            # fill the DMA queues faster.
            tt_ = tpool.tile([P, F], mybir.dt.int32, name="t", tag="t")
            nc.sync.dma_start(out=tt_[:, 0:W], in_=yt[n, j][:, off:off + W])
            ch = []
            for c in range(C):
                tl = chpool.tile([P, F], mybir.dt.float32, name=f"ch{c}", tag=f"ch{c}")
                nc.sync.dma_start(out=tl[:, 0:W], in_=yp[n, c, j][:, off:off + W])
                ch.append(tl)

            # ---- ScalarE: converts (label first: unblocks gt masks) ----
            yf = tpool.tile([P, F], mybir.dt.float16, name="yf", tag="yf")
            nc.scalar.activation(out=yf[:, 0:W], in_=tt_[:, 0:W], func=ACT.Copy)
            chf = []
            for c in range(C):
                tf = chfpool.tile([P, F], mybir.dt.float16, name=f"cf{c}", tag=f"cf{c}")
                nc.scalar.activation(out=tf[:, 0:W], in_=ch[c][:, 0:W], func=ACT.Copy)
                chf.append(tf)

            # ---- DVE: gt masks first (depend only on yf, which converts
            # first — they overlap the channel converts) ----
            gts = []
            for c in range(1, C):
                gt = gtpool.tile([P, F], mybir.dt.float16, name=f"gt{c}", tag=f"gt{c}")
                nc.vector.tensor_scalar(
                    out=gt[:, 0:W], in0=yf[:, 0:W], scalar1=float(c), scalar2=0.0,
                    op0=AL.is_equal, op1=AL.add,
                )
                gts.append(gt)

            # ---- DVE: max tree (fp16 tensor_tensor, 2x) ----
            m01 = mtmp.tile([P, F], mybir.dt.float16, name="m01", tag="m01")
            nc.vector.tensor_max(m01[:, 0:W], chf[0][:, 0:W], chf[1][:, 0:W])
            m23 = mtmp.tile([P, F], mybir.dt.float16, name="m23", tag="m23")
            nc.vector.tensor_max(m23[:, 0:W], chf[2][:, 0:W], chf[3][:, 0:W])
            m45 = mtmp.tile([P, F], mybir.dt.float16, name="m45", tag="m45")
            nc.vector.tensor_max(m45[:, 0:W], chf[4][:, 0:W], chf[5][:, 0:W])
            m67 = mtmp.tile([P, F], mybir.dt.float16, name="m67", tag="m67")
            nc.vector.tensor_max(m67[:, 0:W], chf[6][:, 0:W], chf[7][:, 0:W])
            m0123 = mtmp.tile([P, F], mybir.dt.float16, name="m0123", tag="m01")
            nc.vector.tensor_max(m0123[:, 0:W], m01[:, 0:W], m23[:, 0:W])
            m4567 = mtmp.tile([P, F], mybir.dt.float16, name="m4567", tag="m45")
            nc.vector.tensor_max(m4567[:, 0:W], m45[:, 0:W], m67[:, 0:W])
            m = mpool.tile([P, F], mybir.dt.float16, name="m", tag="m")
            nc.vector.tensor_max(m[:, 0:W], m0123[:, 0:W], m4567[:, 0:W])

            # ---- per class: pred mask (tt is_equal, 2x) ----
            for c in range(1, C):
                px = preds[c - 1]
                nc.vector.tensor_tensor(
                    out=px[:, 0:NS, 1:129], in0=chf[c][:, 0:W], in1=m[:, 0:W],
                    op=AL.is_equal,
                )

            # ---- PE: diag MMs subtile-outer so consecutive MMs hit
            # different psum banks and pipeline; then shared-bank colsums ----
            for s in range(NS):
                for c in range(1, C):
                    nc.tensor.matmul(
                        banks[c - 1][:, 0:129],
                        lhsT=gts[c - 1][:, s * 128:(s + 1) * 128],
                        rhs=preds[c - 1][:, s, 0:129],
                        start=(first and s == 0),
                        stop=(last and s == NS - 1),
                        skip_group_check=True,
                    )
            # colsum MMs in groups of <=4 subtiles; the host sums every psum
            # column, so narrow pieces accumulating only cols [0:gw) is fine
            gstarts = list(range(0, NS, 4))
            for c in range(1, C):
                for gi, g0 in enumerate(gstarts):
                    ng = min(4, NS - g0)
                    nc.tensor.matmul(
                        bbank[0:7, 0:ng * 128],
                        lhsT=ohs[c - 1][:, 0:7],
                        rhs=preds[c - 1][:, g0:g0 + ng, 1:129],
                        start=(first and c == 1 and gi == 0),
                        stop=(last and c == C - 1 and gi == len(gstarts) - 1),
                        skip_group_check=True,
                    )

        # ---- readback: PSUM -> one SBUF tile (copies split DVE/ScalarE,
        # run in parallel) -> single DMA per output ----
        outp = ctx.enter_context(tc.tile_pool(name="out", bufs=1))
        oa = outp.tile([P, 7, 129], mybir.dt.float32, name="oa", tag="oa")
        for c in range(7):
            if c % 2 == 0:
                nc.vector.tensor_copy(out=oa[:, c, :], in_=banks[c][:, 0:129])
            else:
                nc.scalar.copy(out=oa[:, c, :], in_=banks[c][:, 0:129])
        nc.sync.dma_start(out=a_out[:, :, :], in_=oa)
        ob = outp.tile([7, 512], mybir.dt.float32, name="ob", tag="ob")
        nc.scalar.copy(out=ob, in_=bbank[0:7, 0:512])
        nc.sync.dma_start(out=b_out[:, :], in_=ob)

    nc.finalize()
    return nc


def _get_bass():
    global _CACHED_NC
    if _CACHED_NC is None:
        _CACHED_NC = build_bass()
    return _CACHED_NC


def make_in_maps(y_true, y_pred):
    yp = np.ascontiguousarray(np.asarray(y_pred, dtype=np.float32))
    yt = np.ascontiguousarray(np.asarray(y_true, dtype=np.int32))
    in_maps = []
    for i in range(N_CORES):
        yps = np.ascontiguousarray(yp[NB * i: NB * (i + 1)]).reshape(NB, C, NCHUNK, P, F)
        yts = np.ascontiguousarray(yt[NB * i: NB * (i + 1)]).reshape(NB, NCHUNK, P, F)
        in_maps.append({"yp": yps, "yt": yts})
    return in_maps


def epilogue(results):
    """Combine the 8 cores' exact f32 partial sums into the dice mean."""
    tp = np.zeros(7, dtype=np.float64)
    gt_cnt = np.zeros(7, dtype=np.float64)
    pred_cnt = np.zeros(7, dtype=np.float64)
    idx = np.arange(128)
    for r in results:
        a = np.asarray(r["a_out"], dtype=np.float64)   # [128, 7, 129]
        b = np.asarray(r["b_out"], dtype=np.float64)   # [7, 512]
        gt_cnt += a[:, :, 0].sum(axis=0)
        tp += a[idx, :, 1 + idx].sum(axis=0)
        pred_cnt += b.sum(axis=1)

    tp32 = tp.astype(np.float32)
    denom = (pred_cnt + gt_cnt).astype(np.float32)
    eps = np.float32(EPS)
    two = np.float32(2.0)
    dice = (two * tp32 + eps) / (denom + eps)
    return np.asarray(np.mean(dice, dtype=np.float32), dtype=np.float32)


def kernel(**inputs):
    from concourse.bass_utils import run_bass_kernel_spmd

    nc = _get_bass()
    in_maps = make_in_maps(inputs["y_true"], inputs["y_pred"])
    res = run_bass_kernel_spmd(nc, in_maps, core_ids=list(range(N_CORES)))
    return epilogue(res.results)


if __name__ == "__main__":
    rng = np.random.default_rng(0)
    y_true = rng.integers(0, C, size=(16, 512, 512)).astype(np.int32)
    y_pred = rng.standard_normal((16, C, 512, 512)).astype(np.float32)
    out = kernel(y_true=y_true, y_pred=y_pred)
    print("kernel output:", out)

    # numpy oracle
    pred_cls = np.argmax(y_pred, axis=1)
    tp = np.zeros(7); fp = np.zeros(7); fn = np.zeros(7)
    for c in range(1, 8):
        pm = pred_cls == c
        gm = y_true == c
        tp[c-1] = np.sum(pm & gm)
        fp[c-1] = np.sum(pm & ~gm)
        fn[c-1] = np.sum(~pm & gm)
    dice = (2*tp + EPS) / (2*tp + fp + fn + EPS)
    print("numpy oracle:", dice.mean())


# revision 25
# speedup vs baseline: 1.2105x; 1.0182x over previous
"""Trainium2 Bass kernel for DiceLoss (hard-argmax dice, ignore background, mean).

Problem (hardcoded shapes):
  y_true: [16, 512, 512] int32 in [0, 8)
  y_pred: [16, 8, 512, 512] float32
  out   : scalar float32 = mean over classes 1..7 of
          (2*tp + eps) / (2*tp + fp + fn + eps)
        = (2*tp + eps) / (pred_cnt + gt_cnt + eps)

Strategy (8 NeuronCores, data-parallel over batch; 2 images/core):
  - Streams image planes as [128, 1024] chunks (contiguous HBM DMA).
  - ScalarE: f32->fp16 channel converts + int32->fp16 label convert.
  - DVE (all fp16, 2x/4x perf modes): 7-op pairwise max tree; per class
    pred_c = (ch[c] == m) via tensor_tensor is_equal written into a
    [128, 8, 129] layout whose group-col 0 holds ones; gt_c = (y == c)
    via tensor_scalar is_equal (4x mode, flat [128, 1024]).
  - PE: per (class, chunk, subtile) one matmul
        psum_c[:, 0:129] += gt_s^T @ [ones | pred_s]
    giving gt colsums in col 0 and tp on the shifted diagonal (7 psum
    banks, subtile-outer emission so consecutive matmuls hit different
    banks and pipeline); plus per (class, chunk) 2 one-hot-stationary
    colsum matmuls
        psum_b[0:7, 0:512] += onehot_c^T @ pred(4 groups of 128)
    accumulating pred counts for all 7 classes row-wise in one shared
    8th PSUM bank. Only the very first matmul into a bank sets start
    (start zeroes the whole bank).
  - Host: combines the 8 cores' exact f32 count sums into the dice mean.
"""

import numpy as np

EPS = 1e-05

N_CORES = 8
NB = 2          # batch images per core
C = 8           # classes
P = 128         # SBUF partitions
F = 1024        # free-dim elements per chunk
NCHUNK = 2      # chunks per image plane (512*512 = 2*128*1024)
CHUNKS = NB * NCHUNK
NSUB = F // 128  # 8 subtiles per chunk

_CACHED_NC = None


def build_bass():
    from contextlib import ExitStack

    import concourse.bacc as bacc
    import concourse.tile as tile
    from concourse import mybir

    AL = mybir.AluOpType
    ACT = mybir.ActivationFunctionType

    nc = bacc.Bacc(None, target_bir_lowering=False)

    yp = nc.dram_tensor(
        "yp", [NB, C, NCHUNK, P, F], mybir.dt.float32, kind="ExternalInput"
    )
    yt = nc.dram_tensor("yt", [NB, NCHUNK, P, F], mybir.dt.int32, kind="ExternalInput")
    # per class: [128, 129] A-region (col0 = gt colsums, diag = tp);
    # partition-major so one contiguous DMA ships all 7 classes
    a_out = nc.dram_tensor("a_out", [P, 7, 129], mybir.dt.float32, kind="ExternalOutput")
    # pred-count partial colsums: row c-1 = class c
    b_out = nc.dram_tensor("b_out", [7, 512], mybir.dt.float32, kind="ExternalOutput")

    with tile.TileContext(nc) as tc, ExitStack() as ctx:
        chpool = ctx.enter_context(tc.tile_pool(name="ch", bufs=2))
        chfpool = ctx.enter_context(tc.tile_pool(name="chf", bufs=2))
        tpool = ctx.enter_context(tc.tile_pool(name="tt", bufs=3))
        mtmp = ctx.enter_context(tc.tile_pool(name="mtmp", bufs=2))
        mpool = ctx.enter_context(tc.tile_pool(name="mx", bufs=2))
        gtpool = ctx.enter_context(tc.tile_pool(name="gt", bufs=2))
        # two fixed pred-tile sets (manual double buffer, ones cols set once)
        predpA = ctx.enter_context(tc.tile_pool(name="pdA", bufs=1))
        predpB = ctx.enter_context(tc.tile_pool(name="pdB", bufs=1))
        onesp = ctx.enter_context(tc.tile_pool(name="on", bufs=1))
        psump = ctx.enter_context(tc.tile_pool(name="psum", bufs=1, space="PSUM"))

        banks = [
            psump.tile([P, 512], mybir.dt.float32, name=f"bk{c}", tag=f"bk{c}")
            for c in range(7)
        ]
        bbank = psump.tile([P, 512], mybir.dt.float32, name="bb", tag="bb")

        # one-hot stationary columns: ohs[c-1][:, c-1] = 1 for class c
        ohs = []
        for c in range(7):
            t = onesp.tile([P, 7], mybir.dt.float16, name=f"oh{c}")
            nc.vector.memset(t, 0.0)
            nc.vector.memset(t[:, c:c + 1], 1.0)
            ohs.append(t)

        predA = [
            predpA.tile([P, NSUB, 129], mybir.dt.float16, name=f"pA{c}", tag=f"pA{c}")
            for c in range(1, C)
        ]
        predB = [
            predpB.tile([P, NSUB, 129], mybir.dt.float16, name=f"pB{c}", tag=f"pB{c}")
            for c in range(1, C)
        ]
        for t in predA + predB:
            nc.vector.memset(t[:, :, 0:1], 1.0)

        # Work list: (image, plane-chunk, col offset, width). The final plane
        # is processed in two half-width pieces so the last piece's serial
        # convert -> tree -> mask -> matmul chain (the kernel's tail, which
        # runs after the DMA window closes) is half as long.
        pieces = [
            (0, 0, 0, F), (0, 1, 0, F), (1, 0, 0, F),
            (1, 1, 0, F // 2), (1, 1, F // 2, F // 2),
        ]
        for pi, (n, j, off, W) in enumerate(pieces):
            first = pi == 0
            last = pi == len(pieces) - 1
            NS = W // 128
            preds = predA if pi % 2 == 0 else predB

            # Label DMA first: the label convert heads ScalarE's queue and
            # the gt masks head DVE's queue, so yt must never be the
            # straggler transfer. Channel dispatch split sync/gpsimd to
            # fill the DMA queues faster.
            tt_ = tpool.tile([P, F], mybir.dt.int32, name="t", tag="t")
            nc.sync.dma_start(out=tt_[:, 0:W], in_=yt[n, j][:, off:off + W])
            ch = []
            for c in range(C):
                tl = chpool.tile([P, F], mybir.dt.float32, name=f"ch{c}", tag=f"ch{c}")
                nc.sync.dma_start(out=tl[:, 0:W], in_=yp[n, c, j][:, off:off + W])
                ch.append(tl)

            # ---- ScalarE: converts (label first: unblocks gt masks) ----
            yf = tpool.tile([P, F], mybir.dt.float16, name="yf", tag="yf")
            nc.scalar.activation(out=yf[:, 0:W], in_=tt_[:, 0:W], func=ACT.Copy)
            chf = []
            for c in range(C):
                tf = chfpool.tile([P, F], mybir.dt.float16, name=f"cf{c}", tag=f"cf{c}")
                nc.scalar.activation(out=tf[:, 0:W], in_=ch[c][:, 0:W], func=ACT.Copy)
                chf.append(tf)

            # ---- DVE: gt masks first (depend only on yf, which converts
            # first — they overlap the channel converts) ----
            gts = []
            for c in range(1, C):
                gt = gtpool.tile([P, F], mybir.dt.float16, name=f"gt{c}", tag=f"gt{c}")
                nc.vector.tensor_scalar(
                    out=gt[:, 0:W], in0=yf[:, 0:W], scalar1=float(c), scalar2=0.0,
                    op0=AL.is_equal, op1=AL.add,
                )
                gts.append(gt)

            # ---- DVE: max tree (fp16 tensor_tensor, 2x) ----
            m01 = mtmp.tile([P, F], mybir.dt.float16, name="m01", tag="m01")
            nc.vector.tensor_max(m01[:, 0:W], chf[0][:, 0:W], chf[1][:, 0:W])
            m23 = mtmp.tile([P, F], mybir.dt.float16, name="m23", tag="m23")
            nc.vector.tensor_max(m23[:, 0:W], chf[2][:, 0:W], chf[3][:, 0:W])
            m45 = mtmp.tile([P, F], mybir.dt.float16, name="m45", tag="m45")
            nc.vector.tensor_max(m45[:, 0:W], chf[4][:, 0:W], chf[5][:, 0:W])
            m67 = mtmp.tile([P, F], mybir.dt.float16, name="m67", tag="m67")
            nc.vector.tensor_max(m67[:, 0:W], chf[6][:, 0:W], chf[7][:, 0:W])
            m0123 = mtmp.tile([P, F], mybir.dt.float16, name="m0123", tag="m01")
            nc.vector.tensor_max(m0123[:, 0:W], m01[:, 0:W], m23[:, 0:W])
            m4567 = mtmp.tile([P, F], mybir.dt.float16, name="m4567", tag="m45")
            nc.vector.tensor_max(m4567[:, 0:W], m45[:, 0:W], m67[:, 0:W])
            m = mpool.tile([P, F], mybir.dt.float16, name="m", tag="m")
            nc.vector.tensor_max(m[:, 0:W], m0123[:, 0:W], m4567[:, 0:W])

            # ---- per class: pred mask (tt is_equal, 2x) ----
            for c in range(1, C):
                px = preds[c - 1]
                nc.vector.tensor_tensor(
                    out=px[:, 0:NS, 1:129], in0=chf[c][:, 0:W], in1=m[:, 0:W],
                    op=AL.is_equal,
                )

            # ---- PE: diag MMs subtile-outer so consecutive MMs hit
            # different psum banks and pipeline; then shared-bank colsums ----
            for s in range(NS):
                for c in range(1, C):
                    nc.tensor.matmul(
                        banks[c - 1][:, 0:129],
                        lhsT=gts[c - 1][:, s * 128:(s + 1) * 128],
                        rhs=preds[c - 1][:, s, 0:129],
                        start=(first and s == 0),
                        stop=(last and s == NS - 1),
                        skip_group_check=True,
                    )
            # colsum MMs in groups of <=4 subtiles; the host sums every psum
            # column, so narrow pieces accumulating only cols [0:gw) is fine
            gstarts = list(range(0, NS, 4))
            for c in range(1, C):
                for gi, g0 in enumerate(gstarts):
                    ng = min(4, NS - g0)
                    nc.tensor.matmul(
                        bbank[0:7, 0:ng * 128],
                        lhsT=ohs[c - 1][:, 0:7],
                        rhs=preds[c - 1][:, g0:g0 + ng, 1:129],
                        start=(first and c == 1 and gi == 0),
                        stop=(last and c == C - 1 and gi == len(gstarts) - 1),
                        skip_group_check=True,
                    )

        # ---- readback: PSUM -> one SBUF tile (copies split DVE/ScalarE,
        # run in parallel) -> single DMA per output ----
        outp = ctx.enter_context(tc.tile_pool(name="out", bufs=1))
        oa = outp.tile([P, 7, 129], mybir.dt.float32, name="oa", tag="oa")
        for c in range(7):
            if c % 2 == 0:
                nc.vector.tensor_copy(out=oa[:, c, :], in_=banks[c][:, 0:129])
            else:
                nc.scalar.copy(out=oa[:, c, :], in_=banks[c][:, 0:129])
        nc.sync.dma_start(out=a_out[:, :, :], in_=oa)
        ob = outp.tile([7, 512], mybir.dt.float32, name="ob", tag="ob")
        nc.scalar.copy(out=ob, in_=bbank[0:7, 0:512])
        nc.sync.dma_start(out=b_out[:, :], in_=ob)

    nc.finalize()
    return nc


def _get_bass():
    global _CACHED_NC
    if _CACHED_NC is None:
        _CACHED_NC = build_bass()
    return _CACHED_NC


def make_in_maps(y_true, y_pred):
    yp = np.ascontiguousarray(np.asarray(y_pred, dtype=np.float32))
    yt = np.ascontiguousarray(np.asarray(y_true, dtype=np.int32))
    in_maps = []
    for i in range(N_CORES):
        yps = np.ascontiguousarray(yp[NB * i: NB * (i + 1)]).reshape(NB, C, NCHUNK, P, F)
        yts = np.ascontiguousarray(yt[NB * i: NB * (i + 1)]).reshape(NB, NCHUNK, P, F)
        in_maps.append({"yp": yps, "yt": yts})
    return in_maps


def epilogue(results):
    """Combine the 8 cores' exact f32 partial sums into the dice mean."""
    tp = np.zeros(7, dtype=np.float64)
    gt_cnt = np.zeros(7, dtype=np.float64)
    pred_cnt = np.zeros(7, dtype=np.float64)
    idx = np.arange(128)
    for r in results:
        a = np.asarray(r["a_out"], dtype=np.float64)   # [128, 7, 129]
        b = np.asarray(r["b_out"], dtype=np.float64)   # [7, 512]
        gt_cnt += a[:, :, 0].sum(axis=0)
        tp += a[idx, :, 1 + idx].sum(axis=0)
        pred_cnt += b.sum(axis=1)

    tp32 = tp.astype(np.float32)
    denom = (pred_cnt + gt_cnt).astype(np.float32)
    eps = np.float32(EPS)
    two = np.float32(2.0)
    dice = (two * tp32 + eps) / (denom + eps)
    return np.asarray(np.mean(dice, dtype=np.float32), dtype=np.float32)


def kernel(**inputs):
    from concourse.bass_utils import run_bass_kernel_spmd

    nc = _get_bass()
    in_maps = make_in_maps(inputs["y_true"], inputs["y_pred"])
    res = run_bass_kernel_spmd(nc, in_maps, core_ids=list(range(N_CORES)))
    return epilogue(res.results)


if __name__ == "__main__":
    rng = np.random.default_rng(0)
    y_true = rng.integers(0, C, size=(16, 512, 512)).astype(np.int32)
    y_pred = rng.standard_normal((16, C, 512, 512)).astype(np.float32)
    out = kernel(y_true=y_true, y_pred=y_pred)
    print("kernel output:", out)

    # numpy oracle
    pred_cls = np.argmax(y_pred, axis=1)
    tp = np.zeros(7); fp = np.zeros(7); fn = np.zeros(7)
    for c in range(1, 8):
        pm = pred_cls == c
        gm = y_true == c
        tp[c-1] = np.sum(pm & gm)
        fp[c-1] = np.sum(pm & ~gm)
        fn[c-1] = np.sum(~pm & gm)
    dice = (2*tp + EPS) / (2*tp + fp + fn + EPS)
    print("numpy oracle:", dice.mean())


# revision 26
# speedup vs baseline: 1.2785x; 1.0562x over previous
"""Trainium2 Bass kernel for DiceLoss (hard-argmax dice, ignore background, mean).

Problem (hardcoded shapes):
  y_true: [16, 512, 512] int32 in [0, 8)
  y_pred: [16, 8, 512, 512] float32
  out   : scalar float32 = mean over classes 1..7 of
          (2*tp + eps) / (2*tp + fp + fn + eps)
        = (2*tp + eps) / (pred_cnt + gt_cnt + eps)

Strategy (8 NeuronCores, data-parallel over batch; 2 images/core):
  - Streams image planes as [128, 1024] chunks (contiguous HBM DMA).
  - ScalarE: f32->fp16 channel converts + int32->fp16 label convert.
  - DVE (all fp16, 2x/4x perf modes): 7-op pairwise max tree; per class
    pred_c = (ch[c] == m) via tensor_tensor is_equal written into a
    [128, 8, 129] layout whose group-col 0 holds ones; gt_c = (y == c)
    via tensor_scalar is_equal (4x mode, flat [128, 1024]).
  - PE: per (class, chunk, subtile) one matmul
        psum_c[:, 0:129] += gt_s^T @ [ones | pred_s]
    giving gt colsums in col 0 and tp on the shifted diagonal (7 psum
    banks, subtile-outer emission so consecutive matmuls hit different
    banks and pipeline); plus per (class, chunk) 2 one-hot-stationary
    colsum matmuls
        psum_b[0:7, 0:512] += onehot_c^T @ pred(4 groups of 128)
    accumulating pred counts for all 7 classes row-wise in one shared
    8th PSUM bank. Only the very first matmul into a bank sets start
    (start zeroes the whole bank).
  - Host: combines the 8 cores' exact f32 count sums into the dice mean.
"""

import numpy as np

EPS = 1e-05

N_CORES = 8
NB = 2          # batch images per core
C = 8           # classes
P = 128         # SBUF partitions
F = 1024        # free-dim elements per chunk
NCHUNK = 2      # chunks per image plane (512*512 = 2*128*1024)
CHUNKS = NB * NCHUNK
NSUB = F // 128  # 8 subtiles per chunk

_CACHED_NC = None


def build_bass():
    from contextlib import ExitStack

    import concourse.bacc as bacc
    import concourse.tile as tile
    from concourse import mybir

    AL = mybir.AluOpType
    ACT = mybir.ActivationFunctionType

    nc = bacc.Bacc(None, target_bir_lowering=False)

    yp = nc.dram_tensor(
        "yp", [NB, C, NCHUNK, P, F], mybir.dt.float32, kind="ExternalInput"
    )
    yt = nc.dram_tensor("yt", [NB, NCHUNK, P, F], mybir.dt.int32, kind="ExternalInput")
    # per class: [128, 129] A-region (col0 = gt colsums, diag = tp);
    # partition-major so one contiguous DMA ships all 7 classes
    a_out = nc.dram_tensor("a_out", [P, 7, 129], mybir.dt.float32, kind="ExternalOutput")
    # pred-count partial colsums: row c-1 = class c
    b_out = nc.dram_tensor("b_out", [7, 512], mybir.dt.float32, kind="ExternalOutput")

    with tile.TileContext(nc) as tc, ExitStack() as ctx:
        chpool = ctx.enter_context(tc.tile_pool(name="ch", bufs=2))
        chfpool = ctx.enter_context(tc.tile_pool(name="chf", bufs=2))
        tpool = ctx.enter_context(tc.tile_pool(name="tt", bufs=2))
        mtmp = ctx.enter_context(tc.tile_pool(name="mtmp", bufs=2))
        mpool = ctx.enter_context(tc.tile_pool(name="mx", bufs=2))
        gtpool = ctx.enter_context(tc.tile_pool(name="gt", bufs=2))
        # two fixed pred-tile sets (manual double buffer, ones cols set once)
        predpA = ctx.enter_context(tc.tile_pool(name="pdA", bufs=1))
        predpB = ctx.enter_context(tc.tile_pool(name="pdB", bufs=1))
        onesp = ctx.enter_context(tc.tile_pool(name="on", bufs=1))
        psump = ctx.enter_context(tc.tile_pool(name="psum", bufs=1, space="PSUM"))

        banks = [
            psump.tile([P, 512], mybir.dt.float32, name=f"bk{c}", tag=f"bk{c}")
            for c in range(7)
        ]
        bbank = psump.tile([P, 512], mybir.dt.float32, name="bb", tag="bb")

        # one-hot stationary columns: ohs[c-1][:, c-1] = 1 for class c
        ohs = []
        for c in range(7):
            t = onesp.tile([P, 7], mybir.dt.float16, name=f"oh{c}")
            nc.vector.memset(t, 0.0)
            nc.vector.memset(t[:, c:c + 1], 1.0)
            ohs.append(t)

        predA = [
            predpA.tile([P, NSUB, 129], mybir.dt.float16, name=f"pA{c}", tag=f"pA{c}")
            for c in range(1, C)
        ]
        predB = [
            predpB.tile([P, NSUB, 129], mybir.dt.float16, name=f"pB{c}", tag=f"pB{c}")
            for c in range(1, C)
        ]
        for t in predA + predB:
            nc.vector.memset(t[:, :, 0:1], 1.0)

        # Work list: (image, plane-chunk, col offset, width). The final plane
        # is processed in two half-width pieces so the last piece's serial
        # convert -> tree -> mask -> matmul chain (the kernel's tail, which
        # runs after the DMA window closes) is half as long.
        pieces = [
            (0, 0, 0, F), (0, 1, 0, F), (1, 0, 0, F),
            (1, 1, 0, F // 2), (1, 1, F // 2, F // 2),
        ]
        for pi, (n, j, off, W) in enumerate(pieces):
            first = pi == 0
            last = pi == len(pieces) - 1
            NS = W // 128
            preds = predA if pi % 2 == 0 else predB

            # Label DMA first: the label convert heads ScalarE's queue and
            # the gt masks head DVE's queue, so yt must never be the
            # straggler transfer. Channel dispatch split sync/gpsimd to
            # fill the DMA queues faster.
            tt_ = tpool.tile([P, F], mybir.dt.int32, name="t", tag="t")
            nc.sync.dma_start(out=tt_[:, 0:W], in_=yt[n, j][:, off:off + W])
            ch = []
            for c in range(C):
                tl = chpool.tile([P, F], mybir.dt.float32, name=f"ch{c}", tag=f"ch{c}")
                nc.sync.dma_start(out=tl[:, 0:W], in_=yp[n, c, j][:, off:off + W])
                ch.append(tl)

            # ---- ScalarE: converts (label first: unblocks gt masks) ----
            yf = tpool.tile([P, F], mybir.dt.float16, name="yf", tag="yf")
            nc.scalar.activation(out=yf[:, 0:W], in_=tt_[:, 0:W], func=ACT.Copy)
            chf = []
            for c in range(C):
                tf = chfpool.tile([P, F], mybir.dt.float16, name=f"cf{c}", tag=f"cf{c}")
                nc.scalar.activation(out=tf[:, 0:W], in_=ch[c][:, 0:W], func=ACT.Copy)
                chf.append(tf)

            # ---- DVE: gt masks first (depend only on yf, which converts
            # first — they overlap the channel converts) ----
            gts = []
            for c in range(1, C):
                gt = gtpool.tile([P, F], mybir.dt.float16, name=f"gt{c}", tag=f"gt{c}")
                nc.vector.tensor_scalar(
                    out=gt[:, 0:W], in0=yf[:, 0:W], scalar1=float(c), scalar2=0.0,
                    op0=AL.is_equal, op1=AL.add,
                )
                gts.append(gt)

            # ---- DVE: max tree (fp16 tensor_tensor, 2x) ----
            m01 = mtmp.tile([P, F], mybir.dt.float16, name="m01", tag="m01")
            nc.vector.tensor_max(m01[:, 0:W], chf[0][:, 0:W], chf[1][:, 0:W])
            m23 = mtmp.tile([P, F], mybir.dt.float16, name="m23", tag="m23")
            nc.vector.tensor_max(m23[:, 0:W], chf[2][:, 0:W], chf[3][:, 0:W])
            m45 = mtmp.tile([P, F], mybir.dt.float16, name="m45", tag="m45")
            nc.vector.tensor_max(m45[:, 0:W], chf[4][:, 0:W], chf[5][:, 0:W])
            m67 = mtmp.tile([P, F], mybir.dt.float16, name="m67", tag="m67")
            nc.vector.tensor_max(m67[:, 0:W], chf[6][:, 0:W], chf[7][:, 0:W])
            m0123 = mtmp.tile([P, F], mybir.dt.float16, name="m0123", tag="m01")
            nc.vector.tensor_max(m0123[:, 0:W], m01[:, 0:W], m23[:, 0:W])
            m4567 = mtmp.tile([P, F], mybir.dt.float16, name="m4567", tag="m45")
            nc.vector.tensor_max(m4567[:, 0:W], m45[:, 0:W], m67[:, 0:W])
            m = mpool.tile([P, F], mybir.dt.float16, name="m", tag="m")
            nc.vector.tensor_max(m[:, 0:W], m0123[:, 0:W], m4567[:, 0:W])

            # ---- per class: pred mask (tt is_equal, 2x) ----
            for c in range(1, C):
                px = preds[c - 1]
                nc.vector.tensor_tensor(
                    out=px[:, 0:NS, 1:129], in0=chf[c][:, 0:W], in1=m[:, 0:W],
                    op=AL.is_equal,
                )

            # ---- PE: diag MMs subtile-outer so consecutive MMs hit
            # different psum banks and pipeline; then shared-bank colsums ----
            for s in range(NS):
                for c in range(1, C):
                    nc.tensor.matmul(
                        banks[c - 1][:, 0:129],
                        lhsT=gts[c - 1][:, s * 128:(s + 1) * 128],
                        rhs=preds[c - 1][:, s, 0:129],
                        start=(first and s == 0),
                        stop=(last and s == NS - 1),
                        skip_group_check=True,
                    )
            # colsum MMs in groups of <=4 subtiles; the host sums every psum
            # column, so narrow pieces accumulating only cols [0:gw) is fine
            gstarts = list(range(0, NS, 4))
            for c in range(1, C):
                for gi, g0 in enumerate(gstarts):
                    ng = min(4, NS - g0)
                    nc.tensor.matmul(
                        bbank[0:7, 0:ng * 128],
                        lhsT=ohs[c - 1][:, 0:7],
                        rhs=preds[c - 1][:, g0:g0 + ng, 1:129],
                        start=(first and c == 1 and gi == 0),
                        stop=(last and c == C - 1 and gi == len(gstarts) - 1),
                        skip_group_check=True,
                    )

        # ---- readback: PSUM -> one SBUF tile (copies split DVE/ScalarE,
        # run in parallel) -> single DMA per output ----
        outp = ctx.enter_context(tc.tile_pool(name="out", bufs=1))
        oa = outp.tile([P, 7, 129], mybir.dt.float32, name="oa", tag="oa")
        for c in range(7):
            if c % 2 == 0:
                nc.vector.tensor_copy(out=oa[:, c, :], in_=banks[c][:, 0:129])
            else:
                nc.scalar.copy(out=oa[:, c, :], in_=banks[c][:, 0:129])
        nc.sync.dma_start(out=a_out[:, :, :], in_=oa)
        ob = outp.tile([7, 512], mybir.dt.float32, name="ob", tag="ob")
        nc.scalar.copy(out=ob, in_=bbank[0:7, 0:512])
        nc.sync.dma_start(out=b_out[:, :], in_=ob)

    nc.finalize()
    return nc


def _get_bass():
    global _CACHED_NC
    if _CACHED_NC is None:
        _CACHED_NC = build_bass()
    return _CACHED_NC


def make_in_maps(y_true, y_pred):
    yp = np.ascontiguousarray(np.asarray(y_pred, dtype=np.float32))
    yt = np.ascontiguousarray(np.asarray(y_true, dtype=np.int32))
    in_maps = []
    for i in range(N_CORES):
        yps = np.ascontiguousarray(yp[NB * i: NB * (i + 1)]).reshape(NB, C, NCHUNK, P, F)
        yts = np.ascontiguousarray(yt[NB * i: NB * (i + 1)]).reshape(NB, NCHUNK, P, F)
        in_maps.append({"yp": yps, "yt": yts})
    return in_maps


def epilogue(results):
    """Combine the 8 cores' exact f32 partial sums into the dice mean."""
    tp = np.zeros(7, dtype=np.float64)
    gt_cnt = np.zeros(7, dtype=np.float64)
    pred_cnt = np.zeros(7, dtype=np.float64)
    idx = np.arange(128)
    for r in results:
        a = np.asarray(r["a_out"], dtype=np.float64)   # [128, 7, 129]
        b = np.asarray(r["b_out"], dtype=np.float64)   # [7, 512]
        gt_cnt += a[:, :, 0].sum(axis=0)
        tp += a[idx, :, 1 + idx].sum(axis=0)
        pred_cnt += b.sum(axis=1)

    tp32 = tp.astype(np.float32)
    denom = (pred_cnt + gt_cnt).astype(np.float32)
    eps = np.float32(EPS)
    two = np.float32(2.0)
    dice = (two * tp32 + eps) / (denom + eps)
    return np.asarray(np.mean(dice, dtype=np.float32), dtype=np.float32)


def kernel(**inputs):
    from concourse.bass_utils import run_bass_kernel_spmd

    nc = _get_bass()
    in_maps = make_in_maps(inputs["y_true"], inputs["y_pred"])
    res = run_bass_kernel_spmd(nc, in_maps, core_ids=list(range(N_CORES)))
    return epilogue(res.results)


if __name__ == "__main__":
    rng = np.random.default_rng(0)
    y_true = rng.integers(0, C, size=(16, 512, 512)).astype(np.int32)
    y_pred = rng.standard_normal((16, C, 512, 512)).astype(np.float32)
    out = kernel(y_true=y_true, y_pred=y_pred)
    print("kernel output:", out)

    # numpy oracle
    pred_cls = np.argmax(y_pred, axis=1)
    tp = np.zeros(7); fp = np.zeros(7); fn = np.zeros(7)
    for c in range(1, 8):
        pm = pred_cls == c
        gm = y_true == c
        tp[c-1] = np.sum(pm & gm)
        fp[c-1] = np.sum(pm & ~gm)
        fn[c-1] = np.sum(~pm & gm)
    dice = (2*tp + EPS) / (2*tp + fp + fn + EPS)
    print("numpy oracle:", dice.mean())
